# revision 30
# baseline (speedup 1.0000x reference)
"""Sparse GQA attention (causal + sliding window + global tokens) with LoRA
projections and RoPE, distributed over 8 TRN2 NeuronCores.

Sharding: batch (2) x kv-head-group (4). Core (b, g) computes q heads
4g..4g+3 and kv head g for batch b, producing a partial output-projection
sum; the host adds the 4 group partials per batch.

Host-side preprocessing (exact, linear):
  - LoRA folded into the dense weights: W_eff = W + B @ A.
  - Weights pre-transposed into matmul (lhsT / rhs) layouts, cast to bf16.
  - q/k weight rows permuted to the rotate-half layout (evens then odds)
    so RoPE becomes half-tile multiplies.
  - x transposed to [D, S] per batch (contraction dim on partitions).

Kernel structure (per core):
  Phase A (per 512-token chunk): QKV projections; RoPE via two scalar-
  engine PSUM drains (plain + rotate-half copy, bf16) and three 2x-rate
  DVE ops; V computed transposed then moved to natural [tok, hd] layout
  by PE transposes, with a ones column appended for softmax sums.
  Phase B (scores / PV batches, software-pipelined with A so the PE
  never waits on the RoPE or exp chains): scores are computed
  TRANSPOSED (s^T[k, q]) in 512-col matmuls per key block; the window-
  edge strip is batched across all 4 heads in one matmul; exp on the
  scalar engine; causal/window masking as post-exp {0,1} bf16
  multiplies on DVE; P@V with the ones column yields softmax sums for
  free in PSUM; normalization is fused into the PSUM->SBUF copy; o is
  transposed by one 128-col matmul; output projection per query block
  with per-512-col-piece DMA out.  PSUM is budgeted to exactly 8 banks
  via three shared tile rings.
"""

import os
import sys

import numpy as np

for _p in ("/root/.axon_site", "/root/.axon_site/_ro/trn_rl_repo",
           "/root/.axon_site/_ro/pypackages", "/opt/trn_rl_repo"):
    if os.path.isdir(_p) and _p not in sys.path:
        sys.path.append(_p)

import ml_dtypes
import concourse.bacc as bacc
import concourse.mybir as mybir
import concourse.tile as tile
from concourse.bass_utils import run_bass_kernel_spmd

B, S, D = 2, 2048, 2048
H, KVH, HD = 16, 4, 128
WINDOW, GLOBAL = 512, 64
THETA = 1000000.0
NCORES = 8
GH = H // KVH          # q heads per core
GF = GH * HD           # 512 projection features per core
TOK = 512              # token chunk for projections
NCH = S // TOK         # 4 chunks
NQB = S // 128         # 16 query blocks
NKB = S // 128
INV_SQRT = 1.0 / float(np.sqrt(HD))

F32 = mybir.dt.float32
BF16 = mybir.dt.bfloat16

_PROGRAM = {}


def _emit(nc, t):
    """Emit the per-core Tile program. `t` maps input names to DRAM APs."""
    tc = t["tc"]
    from contextlib import ExitStack

    xTd = t["xt"].ap()      # [NCH,128,16,TOK] chunk-packed
    wqT = t["wqt"].ap()     # [128,16,GF]
    wkT = t["wkt"].ap()     # [128,16,HD]
    wvT = t["wvt"].ap()
    woT = t["wot"].ap()     # [128,GH,D]
    y = t["y"].ap()         # [S, D]

    with ExitStack() as stk:
        singles = stk.enter_context(tc.tile_pool(name="singles", bufs=1))
        persist = stk.enter_context(tc.tile_pool(name="persist", bufs=1))
        wpool = stk.enter_context(tc.tile_pool(name="wpool", bufs=1))
        xpool = stk.enter_context(tc.tile_pool(name="xpool", bufs=2))
        apool = stk.enter_context(tc.tile_pool(name="apool", bufs=2))
        bpool = stk.enter_context(tc.tile_pool(name="bpool", bufs=1))
        spool = stk.enter_context(tc.tile_pool(name="spool", bufs=4))
        cpool = stk.enter_context(tc.tile_pool(name="cpool", bufs=2))
        psa = stk.enter_context(tc.tile_pool(name="psa", bufs=1,
                                             space="PSUM"))
        psb = stk.enter_context(tc.tile_pool(name="psb", bufs=1,
                                             space="PSUM"))

        ident_sb = singles.tile([128, 128], BF16)
        triT_sb = singles.tile([128, 128], BF16)
        edgeT_sb = singles.tile([128, 128], BF16)
        edgegT_sb = singles.tile([128, 128], BF16)

        qT_sb = persist.tile([128, GH, S], BF16)    # [hd, head, tok]
        kT_sb = persist.tile([128, S], BF16)        # [hd, tok]
        v_sb = persist.tile([128, NKB, HD + 1], BF16)  # [tok%128, kblk, hd|1]
        nc.vector.memset(v_sb[:, :, HD:HD + 1], 1.0)

        # DMA order tuned for startup: x chunk 0 (split into 4 pieces)
        # interleaved with the small v/k weights so the first matmuls
        # start as early as possible; cos/sin last (RoPE consumes them
        # well off the PE critical path), x chunk 1 prefetched early.
        xt0 = xpool.tile([128, 16, TOK], BF16, tag="xt")
        xt1 = xpool.tile([128, 16, TOK], BF16, tag="xt")
        wv_sb = wpool.tile([128, 16, HD], BF16)
        wk_sb = wpool.tile([128, 16, HD], BF16)
        wq_sb = wpool.tile([128, 16, GF], BF16)
        nc.sync.dma_start(out=xt0[:, 0:2, :], in_=xTd[0][:, 0:2, :])
        nc.sync.dma_start(out=wv_sb, in_=wvT)
        nc.sync.dma_start(out=xt0[:, 2:4, :], in_=xTd[0][:, 2:4, :])
        nc.sync.dma_start(out=wk_sb, in_=wkT)
        nc.sync.dma_start(out=xt0[:, 4:6, :], in_=xTd[0][:, 4:6, :])
        nc.sync.dma_start(out=xt0[:, 6:8, :], in_=xTd[0][:, 6:8, :])
        nc.sync.dma_start(out=wq_sb[:, 0:8, :], in_=wqT[:, 0:8, :])
        nc.sync.dma_start(out=xt0[:, 8:10, :], in_=xTd[0][:, 8:10, :])
        nc.sync.dma_start(out=xt0[:, 10:12, :], in_=xTd[0][:, 10:12, :])
        nc.sync.dma_start(out=xt0[:, 12:14, :], in_=xTd[0][:, 12:14, :])
        nc.sync.dma_start(out=xt0[:, 14:16, :], in_=xTd[0][:, 14:16, :])
        nc.sync.dma_start(out=wq_sb[:, 8:16, :], in_=wqT[:, 8:16, :])
        nc.sync.dma_start(out=ident_sb, in_=t["ident"].ap())
        nc.sync.dma_start(out=xt1, in_=xTd[1])
        cos_sb = wpool.tile([128, S], BF16)
        nc.sync.dma_start(out=cos_sb, in_=t["cos2t"].ap())
        sin_sb = wpool.tile([128, S], BF16)
        nc.sync.dma_start(out=sin_sb, in_=t["sins2t"].ap())
        nc.sync.dma_start(out=triT_sb, in_=t["triT"].ap())
        nc.sync.dma_start(out=edgeT_sb, in_=t["edgeT"].ap())
        nc.sync.dma_start(out=edgegT_sb, in_=t["edgegT"].ap())
        wo_sb = wpool.tile([128, GH, D], BF16)

        ptk_tiles = {}
        ptg_tiles = {}
        pedge_tiles = {}

        def emit_chunk(c):
            cs = slice(c * TOK, (c + 1) * TOK)
            if c == 0:
                xt = xt0
            elif c == 1:
                xt = xt1
            else:
                xt = xpool.tile([128, 16, TOK], BF16, tag="xt")
                nc.sync.dma_start(out=xt, in_=xTd[c])
            # v first (transposed to natural layout at chunk end),
            # then k, then q0..q3.  k/q get rotate-half RoPE; k is
            # needed first by the following quarter's score matmuls.
            pv = psa.tile([128, TOK], F32, tag="pq", bufs=3)
            for a in range(16):
                nc.tensor.matmul(pv, wv_sb[:, a, :], xt[:, a, :],
                                 start=(a == 0), stop=(a == 15))
            vt = apool.tile([128, TOK], BF16, tag="vt", bufs=2)
            nc.vector.tensor_copy(vt, pv)
            for h in [GH] + list(range(GH)):
                if h == GH:
                    wslc = wk_sb
                    dst = kT_sb[:, cs]
                else:
                    wslc = wq_sb[:, :, h * HD:(h + 1) * HD]
                    dst = qT_sb[:, h, cs]
                pq = psa.tile([128, TOK], F32, tag="pq", bufs=3)
                for a in range(16):
                    nc.tensor.matmul(pq, wslc[:, a, :], xt[:, a, :],
                                     start=(a == 0), stop=(a == 15))
                # RoPE: scalar drains PSUM twice (plain + rotated
                # copy; 1 cyc/elem, cross-partition legal from PSUM),
                # then all DVE ops run on bf16 SBUF at 2x rate.
                pqs = apool.tile([128, TOK], BF16, tag="pqs", bufs=2)
                nc.scalar.copy(pqs, pq)
                pqr = apool.tile([128, TOK], BF16, tag="pqr", bufs=2)
                nc.scalar.copy(pqr[0:64, :], pq[64:128, :])
                nc.scalar.copy(pqr[64:128, :], pq[0:64, :])
                t1 = apool.tile([128, TOK], BF16, tag="t1", bufs=2)
                nc.vector.tensor_mul(t1, pqr, sin_sb[:, cs])
                t2 = apool.tile([128, TOK], BF16, tag="t2", bufs=2)
                nc.vector.tensor_mul(t2, pqs, cos_sb[:, cs])
                nc.vector.tensor_add(dst, t2, t1)
            # v transposes at chunk end: vt long ready, no PE stall
            vtp = psa.tile([128, 4, 128], BF16, tag="vtp", bufs=1)
            for b2 in range(TOK // 128):
                nc.tensor.transpose(vtp[:, b2, :],
                                    vt[:, b2 * 128:(b2 + 1) * 128],
                                    ident_sb)
            nc.vector.tensor_copy(v_sb[:, c * 4:(c + 1) * 4, 0:HD], vtp)

        def emit_quarter(qtr):
            qis = list(range(4 * (qtr - 1), 4 * qtr))
            # global strips first (consumed by this quarter's PV)
            for h in range(GH):
                if qtr >= 2:
                    q0b = max(5, 4 * (qtr - 1))
                    Wg = (4 * qtr - q0b) * 128
                    q0 = q0b * 128
                    gps = psb.tile([128, 512], F32, tag="ps", bufs=4)
                    nc.tensor.matmul(gps[0:64, 0:Wg], kT_sb[:, 0:64],
                                     qT_sb[:, h, q0:q0 + Wg],
                                     start=True, stop=True)
                    ptg = bpool.tile([128, 512], BF16, tag=f"ptg{h}",
                                     bufs=2)
                    nc.scalar.activation(ptg[0:64, 0:Wg], gps[0:64, 0:Wg],
                                         mybir.ActivationFunctionType.Exp,
                                         scale=INV_SQRT)
                    ptg_tiles[h] = (ptg, q0)
            for qi in qis:
                kb = qi
                for h in range(GH):
                    # score tile kb covers q-blocks kb..kb+3 transposed:
                    # s^T[k, q]; diag mask at slice 0.  The q-block kb+4
                    # window-edge strip is computed 4-heads-batched below.
                    nq = min(4, NQB - kb)
                    W = 128 * nq
                    k0 = kb * 128
                    ps = psb.tile([128, 512], F32, tag="ps", bufs=4)
                    nc.tensor.matmul(ps[:, 0:W], kT_sb[:, k0:k0 + 128],
                                     qT_sb[:, h, k0:k0 + W],
                                     start=True, stop=True)
                    ptk = bpool.tile([128, 512], BF16, tag=f"ptk{h}",
                                     bufs=8)
                    nc.scalar.activation(ptk[:, 0:W], ps[:, 0:W],
                                         mybir.ActivationFunctionType.Exp,
                                         scale=INV_SQRT)
                    nc.vector.tensor_mul(ptk[:, 0:128], ptk[:, 0:128],
                                         triT_sb)
                    ptk_tiles[(h, kb)] = ptk
                if kb + 4 <= NQB - 1:
                    # edge strip s^T[k in kb, q in kb+4] for ALL 4 heads
                    # in one 512-col matmul (strided rhs over heads)
                    k0 = kb * 128
                    qe = (kb + 4) * 128
                    pse = psb.tile([128, 512], F32, tag="ps", bufs=4)
                    nc.tensor.matmul(pse, kT_sb[:, k0:k0 + 128],
                                     qT_sb[:, 0:GH, qe:qe + 128],
                                     start=True, stop=True)
                    pedge = bpool.tile([128, 512], BF16, tag="pedge",
                                       bufs=8)
                    nc.scalar.activation(pedge, pse,
                                         mybir.ActivationFunctionType.Exp,
                                         scale=INV_SQRT)
                    msk = edgegT_sb if kb == 0 else edgeT_sb
                    for hh in range(GH):
                        nc.vector.tensor_mul(
                            pedge[:, hh * 128:(hh + 1) * 128],
                            pedge[:, hh * 128:(hh + 1) * 128], msk)
                    pedge_tiles[kb] = pedge
                # PV + output projection for this query block; the
                # exps/masks of the score tiles just emitted drain in
                # the shadow of these pq-ring matmuls.
                ot = cpool.tile([128, GH, 128], BF16, tag="ot", bufs=2)
                for h in range(GH):
                    # po tile layout: [0:129] P@V + sums, [132:260] o^T
                    # (shares the phase-A psum ring: 4 banks, no stall)
                    po = psa.tile([128, TOK], F32, tag="pq", bufs=3)
                    kbs = list(range(max(0, qi - 4), qi + 1))
                    n_mm = len(kbs) + (1 if qi >= 5 else 0)
                    for i, kb in enumerate(kbs):
                        j = qi - kb
                        lhsT = (pedge_tiles[kb][:, h * 128:(h + 1) * 128]
                                if j == 4 else
                                ptk_tiles[(h, kb)][:, j * 128:(j + 1) * 128])
                        nc.tensor.matmul(
                            po[:, 0:HD + 1], lhsT, v_sb[:, kb, 0:HD + 1],
                            start=(i == 0), stop=(i == n_mm - 1))
                    if qi >= 5:
                        ptg, q0 = ptg_tiles[h]
                        off = qi * 128 - q0
                        nc.tensor.matmul(po[:, 0:HD + 1],
                                         ptg[0:64, off:off + 128],
                                         v_sb[0:64, 0, 0:HD + 1],
                                         start=False, stop=True)
                    inv = spool.tile([128, 1], F32, tag="inv")
                    nc.vector.reciprocal(inv, po[:, HD:HD + 1])
                    onat = spool.tile([128, 128], BF16, tag="onat")
                    nc.vector.tensor_scalar_mul(onat, po[:, 0:HD], inv)
                    nc.tensor.matmul(po[:, 132:260], onat, ident_sb,
                                     start=True, stop=True)
                    nc.vector.tensor_copy(ot[:, h, :], po[:, 132:260])
                # output projection for this token block; DMA out each
                # 512-col piece as soon as it is copied (short drain)
                ysb = cpool.tile([128, D], BF16, tag="ysb", bufs=2)
                for cchunk in range(4):
                    ns = slice(cchunk * 512, (cchunk + 1) * 512)
                    py = psb.tile([128, 512], F32, tag="ps", bufs=4)
                    for hh in range(GH):
                        nc.tensor.matmul(py[:, 0:512], ot[:, hh, :],
                                         wo_sb[:, hh, ns],
                                         start=(hh == 0),
                                         stop=(hh == GH - 1))
                    if cchunk % 2 == 0:
                        nc.scalar.copy(ysb[:, ns], py)
                    else:
                        nc.vector.tensor_copy(ysb[:, ns], py)
                    nc.sync.dma_start(
                        out=y[qi * 128:(qi + 1) * 128, ns],
                        in_=ysb[:, ns])

        # Pipeline: score batches (KB) are emitted well after the chunk
        # whose RoPE output they read, and PV batches read only score
        # tiles from a previous KB step — the PE never waits on the
        # DVE RoPE or scalar exp chains.
        nc.sync.dma_start(out=wo_sb, in_=woT)
        emit_chunk(0)
        emit_chunk(1)
        emit_quarter(1)
        emit_chunk(2)
        emit_quarter(2)
        emit_chunk(3)
        emit_quarter(3)
        emit_quarter(4)


def _build_program():
    if "nc" in _PROGRAM:
        return _PROGRAM["nc"]
    nc = bacc.Bacc("TRN2", target_bir_lowering=False, debug=False,
                   num_devices=NCORES)
    t = {}
    t["xt"] = nc.dram_tensor("xt", [NCH, 128, 16, TOK], BF16,
                             kind="ExternalInput")
    t["wqt"] = nc.dram_tensor("wqt", [128, 16, GF], BF16,
                              kind="ExternalInput")
    t["wkt"] = nc.dram_tensor("wkt", [128, 16, HD], BF16,
                              kind="ExternalInput")
    t["wvt"] = nc.dram_tensor("wvt", [128, 16, HD], BF16,
                              kind="ExternalInput")
    t["wot"] = nc.dram_tensor("wot", [128, GH, D], BF16,
                              kind="ExternalInput")
    t["cos2t"] = nc.dram_tensor("cos2t", [128, S], BF16,
                                kind="ExternalInput")
    t["sins2t"] = nc.dram_tensor("sins2t", [128, S], BF16,
                                 kind="ExternalInput")
    t["ident"] = nc.dram_tensor("ident", [128, 128], BF16,
                                kind="ExternalInput")
    t["triT"] = nc.dram_tensor("triT", [128, 128], BF16,
                               kind="ExternalInput")
    t["edgeT"] = nc.dram_tensor("edgeT", [128, 128], BF16,
                                kind="ExternalInput")
    t["edgegT"] = nc.dram_tensor("edgegT", [128, 128], BF16,
                                 kind="ExternalInput")
    t["y"] = nc.dram_tensor("y", [S, D], BF16, kind="ExternalOutput")

    with tile.TileContext(nc) as tc:
        t["tc"] = tc
        _emit(nc, t)
    nc.compile()
    _PROGRAM["nc"] = nc
    return nc


def _host_inputs(x, wq_w, wq_a, wq_b, wk_w, wk_a, wk_b, wv_w, wv_a, wv_b,
                 wo_w, wo_a, wo_b):
    f32 = np.float32
    bf16 = ml_dtypes.bfloat16
    Wq = (wq_w.astype(f32) + wq_b.astype(f32) @ wq_a.astype(f32))
    Wk = (wk_w.astype(f32) + wk_b.astype(f32) @ wk_a.astype(f32))
    Wv = (wv_w.astype(f32) + wv_b.astype(f32) @ wv_a.astype(f32))
    Wo = (wo_w.astype(f32) + wo_b.astype(f32) @ wo_a.astype(f32))

    perm = np.concatenate([np.arange(0, HD, 2), np.arange(1, HD, 2)])
    Wq_p = Wq.reshape(H, HD, D)[:, perm, :].reshape(H * HD, D)
    Wk_p = Wk.reshape(KVH, HD, D)[:, perm, :].reshape(KVH * HD, D)

    j = np.arange(HD // 2, dtype=np.float64)
    inv_freq = 1.0 / THETA ** (2.0 * j / HD)
    tpos = np.arange(S, dtype=np.float64)
    freqs = np.outer(inv_freq, tpos)                      # [64, S]
    cosT = np.cos(freqs)
    sinT = np.sin(freqs)
    cos2t = np.concatenate([cosT, cosT], 0).astype(bf16)
    sins2t = np.concatenate([-sinT, sinT], 0).astype(bf16)

    a = np.arange(128)
    triT = (a[:, None] <= a[None, :]).astype(bf16)
    edgeT = (a[:, None] > a[None, :]).astype(bf16)
    edgegT = ((a[:, None] > a[None, :]) | (a[:, None] < GLOBAL)).astype(bf16)
    ident = np.eye(128, dtype=bf16)

    common = dict(cos2t=cos2t, sins2t=sins2t, triT=triT, edgeT=edgeT,
                  edgegT=edgegT, ident=ident)

    def pack_w(wT, nf):
        # [D, nf] -> [128, 16, nf], partition-contiguous
        return np.ascontiguousarray(
            wT.reshape(16, 128, nf).transpose(1, 0, 2)).astype(bf16)

    NCH_ = S // TOK
    in_maps = []
    for b in range(B):
        xT = x[b].astype(f32).T.astype(bf16)            # [D, S]
        xh = np.ascontiguousarray(
            xT.reshape(16, 128, NCH_, TOK).transpose(2, 1, 0, 3))
        for g in range(KVH):
            woT = Wo[:, GF * g:GF * (g + 1)].T          # [GF, D]
            woh = np.ascontiguousarray(
                woT.reshape(GH, 128, D).transpose(1, 0, 2)).astype(bf16)
            in_maps.append(dict(
                xt=xh,
                wqt=pack_w(Wq_p[GF * g:GF * (g + 1), :].T, GF),
                wkt=pack_w(Wk_p[HD * g:HD * (g + 1), :].T, HD),
                wvt=pack_w(Wv[HD * g:HD * (g + 1), :].T, HD),
                wot=woh,
                **common,
            ))
    return in_maps


def kernel(**inputs):
    nc = _build_program()
    in_maps = _host_inputs(**inputs)
    res = None
    last_err = None
    for _attempt in range(4):
        try:
            res = run_bass_kernel_spmd(nc, in_maps,
                                       core_ids=list(range(NCORES)))
            break
        except Exception as e:  # transient first-exec device hiccups
            last_err = e
            import time as _time
            _time.sleep(3.0 * (_attempt + 1))
    if res is None:
        raise last_err
    out = np.zeros((B, S, D), dtype=np.float32)
    for b in range(B):
        for g in range(KVH):
            out[b] += res.results[b * KVH + g]["y"].astype(np.float32)
    return out


# revision 31
# speedup vs baseline: 1.2573x; 1.2573x over previous
"""Sparse GQA attention (causal + sliding window + global tokens) with LoRA
projections and RoPE, distributed over 8 TRN2 NeuronCores.

Sharding: batch (2) x kv-head-group (4). Core (b, g) computes q heads
4g..4g+3 and kv head g for batch b, producing a partial output-projection
sum; the host adds the 4 group partials per batch.

Host-side preprocessing (exact, linear):
  - LoRA folded into the dense weights: W_eff = W + B @ A.
  - Weights pre-transposed into matmul (lhsT / rhs) layouts, cast to bf16.
  - q/k weight rows permuted to the rotate-half layout (evens then odds)
    so RoPE becomes half-tile multiplies.
  - x transposed to [D, S] per batch (contraction dim on partitions).

Kernel structure (per core):
  Phase A (per 512-token chunk): QKV projections; RoPE via two scalar-
  engine PSUM drains (plain + rotate-half copy, bf16) and three 2x-rate
  DVE ops; V computed transposed then moved to natural [tok, hd] layout
  by PE transposes, with a ones column appended for softmax sums.
  Phase B (scores / PV batches, software-pipelined with A so the PE
  never waits on the RoPE or exp chains): scores are computed
  TRANSPOSED (s^T[k, q]) in 512-col matmuls per key block; the window-
  edge strip is batched across all 4 heads in one matmul; exp on the
  scalar engine; causal/window masking as post-exp {0,1} bf16
  multiplies on DVE; P@V with the ones column yields softmax sums for
  free in PSUM; normalization is fused into the PSUM->SBUF copy; o is
  transposed by one 128-col matmul; output projection per query block
  with per-512-col-piece DMA out.  PSUM is budgeted to exactly 8 banks
  via three shared tile rings.
"""

import os
import sys

import numpy as np

for _p in ("/root/.axon_site", "/root/.axon_site/_ro/trn_rl_repo",
           "/root/.axon_site/_ro/pypackages", "/opt/trn_rl_repo"):
    if os.path.isdir(_p) and _p not in sys.path:
        sys.path.append(_p)

import ml_dtypes
import concourse.bacc as bacc
import concourse.mybir as mybir
import concourse.tile as tile
from concourse.bass_utils import run_bass_kernel_spmd

B, S, D = 2, 2048, 2048
H, KVH, HD = 16, 4, 128
WINDOW, GLOBAL = 512, 64
THETA = 1000000.0
NCORES = 8
GH = H // KVH          # q heads per core
GF = GH * HD           # 512 projection features per core
TOK = 512              # token chunk for projections
NCH = S // TOK         # 4 chunks
NQB = S // 128         # 16 query blocks
NKB = S // 128
INV_SQRT = 1.0 / float(np.sqrt(HD))

F32 = mybir.dt.float32
BF16 = mybir.dt.bfloat16

_PROGRAM = {}


def _emit(nc, t):
    """Emit the per-core Tile program. `t` maps input names to DRAM APs."""
    tc = t["tc"]
    from contextlib import ExitStack

    xTd = t["xt"].ap()      # [NCH,128,16,TOK] chunk-packed
    wqT = t["wqt"].ap()     # [128,16,GF]
    wkT = t["wkt"].ap()     # [128,16,HD]
    wvT = t["wvt"].ap()
    woT = t["wot"].ap()     # [128,GH,D]
    y = t["y"].ap()         # [S, D]

    with ExitStack() as stk:
        singles = stk.enter_context(tc.tile_pool(name="singles", bufs=1))
        persist = stk.enter_context(tc.tile_pool(name="persist", bufs=1))
        wpool = stk.enter_context(tc.tile_pool(name="wpool", bufs=1))
        xpool = stk.enter_context(tc.tile_pool(name="xpool", bufs=2))
        apool = stk.enter_context(tc.tile_pool(name="apool", bufs=2))
        bpool = stk.enter_context(tc.tile_pool(name="bpool", bufs=1))
        spool = stk.enter_context(tc.tile_pool(name="spool", bufs=4))
        cpool = stk.enter_context(tc.tile_pool(name="cpool", bufs=2))
        psa = stk.enter_context(tc.tile_pool(name="psa", bufs=1,
                                             space="PSUM"))
        psb = stk.enter_context(tc.tile_pool(name="psb", bufs=1,
                                             space="PSUM"))

        ident_sb = singles.tile([128, 128], BF16)
        triT_sb = singles.tile([128, 128], BF16)
        edgeT_sb = singles.tile([128, 128], BF16)
        edgegT_sb = singles.tile([128, 128], BF16)

        qT_sb = persist.tile([128, GH, S], BF16)    # [hd, head, tok]
        kT_sb = persist.tile([128, S], BF16)        # [hd, tok]
        v_sb = persist.tile([128, NKB, HD + 1], BF16)  # [tok%128, kblk, hd|1]
        nc.vector.memset(v_sb[:, :, HD:HD + 1], 1.0)

        # DMA order tuned for startup: x chunk 0 (split into 4 pieces)
        # interleaved with the small v/k weights so the first matmuls
        # start as early as possible; cos/sin last (RoPE consumes them
        # well off the PE critical path), x chunk 1 prefetched early.
        xt0 = xpool.tile([128, 16, TOK], BF16, tag="xt")
        xt1 = xpool.tile([128, 16, TOK], BF16, tag="xt")
        wv_sb = wpool.tile([128, 16, HD], BF16)
        wk_sb = wpool.tile([128, 16, HD], BF16)
        wq_sb = wpool.tile([128, 16, GF], BF16)
        nc.sync.dma_start(out=xt0[:, 0:2, :], in_=xTd[0][:, 0:2, :])
        nc.sync.dma_start(out=wv_sb, in_=wvT)
        nc.sync.dma_start(out=xt0[:, 2:4, :], in_=xTd[0][:, 2:4, :])
        nc.sync.dma_start(out=wk_sb, in_=wkT)
        nc.sync.dma_start(out=xt0[:, 4:6, :], in_=xTd[0][:, 4:6, :])
        nc.sync.dma_start(out=xt0[:, 6:8, :], in_=xTd[0][:, 6:8, :])
        nc.sync.dma_start(out=wq_sb[:, 0:8, :], in_=wqT[:, 0:8, :])
        nc.sync.dma_start(out=xt0[:, 8:10, :], in_=xTd[0][:, 8:10, :])
        nc.sync.dma_start(out=xt0[:, 10:12, :], in_=xTd[0][:, 10:12, :])
        nc.sync.dma_start(out=xt0[:, 12:14, :], in_=xTd[0][:, 12:14, :])
        nc.sync.dma_start(out=xt0[:, 14:16, :], in_=xTd[0][:, 14:16, :])
        nc.sync.dma_start(out=wq_sb[:, 8:16, :], in_=wqT[:, 8:16, :])
        nc.sync.dma_start(out=ident_sb, in_=t["ident"].ap())
        nc.sync.dma_start(out=xt1, in_=xTd[1])
        cos_sb = wpool.tile([128, S], BF16)
        nc.sync.dma_start(out=cos_sb, in_=t["cos2t"].ap())
        sin_sb = wpool.tile([128, S], BF16)
        nc.sync.dma_start(out=sin_sb, in_=t["sins2t"].ap())
        nc.sync.dma_start(out=triT_sb, in_=t["triT"].ap())
        nc.sync.dma_start(out=edgeT_sb, in_=t["edgeT"].ap())
        nc.sync.dma_start(out=edgegT_sb, in_=t["edgegT"].ap())
        wo_sb = wpool.tile([128, GH, D], BF16)

        ptk_tiles = {}
        ptg_tiles = {}
        pedge_tiles = {}

        def emit_chunk(c):
            cs = slice(c * TOK, (c + 1) * TOK)
            if c == 0:
                xt = xt0
            elif c == 1:
                xt = xt1
            else:
                xt = xpool.tile([128, 16, TOK], BF16, tag="xt")
                nc.sync.dma_start(out=xt, in_=xTd[c])
            # v first (transposed to natural layout at chunk end),
            # then k, then q0..q3.  k/q get rotate-half RoPE; k is
            # needed first by the following quarter's score matmuls.
            pv = psa.tile([128, TOK], F32, tag="pq", bufs=3)
            for a in range(16):
                nc.tensor.matmul(pv, wv_sb[:, a, :], xt[:, a, :],
                                 start=(a == 0), stop=(a == 15))
            vt = apool.tile([128, TOK], BF16, tag="vt", bufs=2)
            nc.vector.tensor_copy(vt, pv)
            for h in [GH] + list(range(GH)):
                if h == GH:
                    wslc = wk_sb
                    dst = kT_sb[:, cs]
                else:
                    wslc = wq_sb[:, :, h * HD:(h + 1) * HD]
                    dst = qT_sb[:, h, cs]
                pq = psa.tile([128, TOK], F32, tag="pq", bufs=3)
                for a in range(16):
                    nc.tensor.matmul(pq, wslc[:, a, :], xt[:, a, :],
                                     start=(a == 0), stop=(a == 15))
                # RoPE: scalar drains PSUM twice (plain + rotated
                # copy; 1 cyc/elem, cross-partition legal from PSUM),
                # then all DVE ops run on bf16 SBUF at 2x rate.
                pqs = apool.tile([128, TOK], BF16, tag="pqs", bufs=2)
                nc.scalar.copy(pqs, pq)
                pqr = apool.tile([128, TOK], BF16, tag="pqr", bufs=2)
                nc.scalar.copy(pqr[0:64, :], pq[64:128, :])
                nc.scalar.copy(pqr[64:128, :], pq[0:64, :])
                t1 = apool.tile([128, TOK], BF16, tag="t1", bufs=2)
                nc.vector.tensor_mul(t1, pqr, sin_sb[:, cs])
                t2 = apool.tile([128, TOK], BF16, tag="t2", bufs=2)
                nc.vector.tensor_mul(t2, pqs, cos_sb[:, cs])
                nc.vector.tensor_add(dst, t2, t1)
            # v transposes at chunk end: vt long ready, no PE stall
            vtp = psa.tile([128, 4, 128], BF16, tag="vtp", bufs=1)
            for b2 in range(TOK // 128):
                nc.tensor.transpose(vtp[:, b2, :],
                                    vt[:, b2 * 128:(b2 + 1) * 128],
                                    ident_sb)
            nc.vector.tensor_copy(v_sb[:, c * 4:(c + 1) * 4, 0:HD], vtp)

        def emit_quarter(qtr, part):
            qis = list(range(4 * (qtr - 1), 4 * qtr))
            if part == 'pv':
                for qi in qis:
                    emit_pv_unit(qi)
                return
            # global strips first (consumed by this quarter's PV)
            for h in range(GH):
                if qtr >= 2:
                    q0b = max(5, 4 * (qtr - 1))
                    Wg = (4 * qtr - q0b) * 128
                    q0 = q0b * 128
                    gps = psb.tile([128, 512], F32, tag="ps", bufs=4)
                    nc.tensor.matmul(gps[0:64, 0:Wg], kT_sb[:, 0:64],
                                     qT_sb[:, h, q0:q0 + Wg],
                                     start=True, stop=True)
                    ptg = bpool.tile([128, 512], BF16, tag=f"ptg{h}",
                                     bufs=2)
                    nc.scalar.activation(ptg[0:64, 0:Wg], gps[0:64, 0:Wg],
                                         mybir.ActivationFunctionType.Exp,
                                         scale=INV_SQRT)
                    ptg_tiles[h] = (ptg, q0)
            for qi in qis:
                kb = qi
                for h in range(GH):
                    # score tile kb covers q-blocks kb..kb+3 transposed:
                    # s^T[k, q]; diag mask at slice 0.  The q-block kb+4
                    # window-edge strip is computed 4-heads-batched below.
                    nq = min(4, NQB - kb)
                    W = 128 * nq
                    k0 = kb * 128
                    ps = psb.tile([128, 512], F32, tag="ps", bufs=4)
                    nc.tensor.matmul(ps[:, 0:W], kT_sb[:, k0:k0 + 128],
                                     qT_sb[:, h, k0:k0 + W],
                                     start=True, stop=True)
                    ptk = bpool.tile([128, 512], BF16, tag=f"ptk{h}",
                                     bufs=8)
                    nc.scalar.activation(ptk[:, 0:W], ps[:, 0:W],
                                         mybir.ActivationFunctionType.Exp,
                                         scale=INV_SQRT)
                    nc.vector.tensor_mul(ptk[:, 0:128], ptk[:, 0:128],
                                         triT_sb)
                    ptk_tiles[(h, kb)] = ptk
                if kb + 4 <= NQB - 1:
                    # edge strip s^T[k in kb, q in kb+4] for ALL 4 heads
                    # in one 512-col matmul (strided rhs over heads)
                    k0 = kb * 128
                    qe = (kb + 4) * 128
                    pse = psb.tile([128, 512], F32, tag="ps", bufs=4)
                    nc.tensor.matmul(pse, kT_sb[:, k0:k0 + 128],
                                     qT_sb[:, 0:GH, qe:qe + 128],
                                     start=True, stop=True)
                    pedge = bpool.tile([128, 512], BF16, tag="pedge",
                                       bufs=8)
                    nc.scalar.activation(pedge, pse,
                                         mybir.ActivationFunctionType.Exp,
                                         scale=INV_SQRT)
                    msk = edgegT_sb if kb == 0 else edgeT_sb
                    for hh in range(GH):
                        nc.vector.tensor_mul(
                            pedge[:, hh * 128:(hh + 1) * 128],
                            pedge[:, hh * 128:(hh + 1) * 128], msk)
                    pedge_tiles[kb] = pedge

        def emit_pv_unit(qi):
                # PV + output projection for this query block
                ot = cpool.tile([128, GH, 128], BF16, tag="ot", bufs=2)
                for h in range(GH):
                    # po tile layout: [0:129] P@V + sums, [132:260] o^T
                    # (shares the phase-A psum ring: 4 banks, no stall)
                    po = psa.tile([128, TOK], F32, tag="pq", bufs=3)
                    kbs = list(range(max(0, qi - 4), qi + 1))
                    n_mm = len(kbs) + (1 if qi >= 5 else 0)
                    for i, kb in enumerate(kbs):
                        j = qi - kb
                        lhsT = (pedge_tiles[kb][:, h * 128:(h + 1) * 128]
                                if j == 4 else
                                ptk_tiles[(h, kb)][:, j * 128:(j + 1) * 128])
                        nc.tensor.matmul(
                            po[:, 0:HD + 1], lhsT, v_sb[:, kb, 0:HD + 1],
                            start=(i == 0), stop=(i == n_mm - 1))
                    if qi >= 5:
                        ptg, q0 = ptg_tiles[h]
                        off = qi * 128 - q0
                        nc.tensor.matmul(po[:, 0:HD + 1],
                                         ptg[0:64, off:off + 128],
                                         v_sb[0:64, 0, 0:HD + 1],
                                         start=False, stop=True)
                    inv = spool.tile([128, 1], F32, tag="inv")
                    nc.vector.reciprocal(inv, po[:, HD:HD + 1])
                    onat = spool.tile([128, 128], BF16, tag="onat")
                    nc.vector.tensor_scalar_mul(onat, po[:, 0:HD], inv)
                    nc.tensor.matmul(po[:, 132:260], onat, ident_sb,
                                     start=True, stop=True)
                    nc.vector.tensor_copy(ot[:, h, :], po[:, 132:260])
                # output projection for this token block; DMA out each
                # 512-col piece as soon as it is copied (short drain)
                ysb = cpool.tile([128, D], BF16, tag="ysb", bufs=2)
                for cchunk in range(4):
                    ns = slice(cchunk * 512, (cchunk + 1) * 512)
                    py = psb.tile([128, 512], F32, tag="ps", bufs=4)
                    for hh in range(GH):
                        nc.tensor.matmul(py[:, 0:512], ot[:, hh, :],
                                         wo_sb[:, hh, ns],
                                         start=(hh == 0),
                                         stop=(hh == GH - 1))
                    if cchunk % 2 == 0:
                        nc.scalar.copy(ysb[:, ns], py)
                    else:
                        nc.vector.tensor_copy(ysb[:, ns], py)
                    nc.sync.dma_start(
                        out=y[qi * 128:(qi + 1) * 128, ns],
                        in_=ysb[:, ns])

        # Pipeline: score batches (KB) are emitted well after the chunk
        # whose RoPE output they read, and PV batches read only score
        # tiles from a previous KB step — the PE never waits on the
        # DVE RoPE or scalar exp chains.
        emit_chunk(0)
        emit_chunk(1)
        emit_quarter(1, 'kb')
        emit_chunk(2)
        nc.sync.dma_start(out=wo_sb, in_=woT)
        emit_quarter(1, 'pv')
        emit_quarter(2, 'kb')
        emit_chunk(3)
        emit_quarter(2, 'pv')
        emit_quarter(3, 'kb')
        emit_quarter(3, 'pv')
        emit_quarter(4, 'kb')
        emit_quarter(4, 'pv')


def _build_program():
    if "nc" in _PROGRAM:
        return _PROGRAM["nc"]
    nc = bacc.Bacc("TRN2", target_bir_lowering=False, debug=False,
                   num_devices=NCORES)
    t = {}
    t["xt"] = nc.dram_tensor("xt", [NCH, 128, 16, TOK], BF16,
                             kind="ExternalInput")
    t["wqt"] = nc.dram_tensor("wqt", [128, 16, GF], BF16,
                              kind="ExternalInput")
    t["wkt"] = nc.dram_tensor("wkt", [128, 16, HD], BF16,
                              kind="ExternalInput")
    t["wvt"] = nc.dram_tensor("wvt", [128, 16, HD], BF16,
                              kind="ExternalInput")
    t["wot"] = nc.dram_tensor("wot", [128, GH, D], BF16,
                              kind="ExternalInput")
    t["cos2t"] = nc.dram_tensor("cos2t", [128, S], BF16,
                                kind="ExternalInput")
    t["sins2t"] = nc.dram_tensor("sins2t", [128, S], BF16,
                                 kind="ExternalInput")
    t["ident"] = nc.dram_tensor("ident", [128, 128], BF16,
                                kind="ExternalInput")
    t["triT"] = nc.dram_tensor("triT", [128, 128], BF16,
                               kind="ExternalInput")
    t["edgeT"] = nc.dram_tensor("edgeT", [128, 128], BF16,
                                kind="ExternalInput")
    t["edgegT"] = nc.dram_tensor("edgegT", [128, 128], BF16,
                                 kind="ExternalInput")
    t["y"] = nc.dram_tensor("y", [S, D], BF16, kind="ExternalOutput")

    with tile.TileContext(nc) as tc:
        t["tc"] = tc
        _emit(nc, t)
    nc.compile()
    _PROGRAM["nc"] = nc
    return nc


def _host_inputs(x, wq_w, wq_a, wq_b, wk_w, wk_a, wk_b, wv_w, wv_a, wv_b,
                 wo_w, wo_a, wo_b):
    f32 = np.float32
    bf16 = ml_dtypes.bfloat16
    Wq = (wq_w.astype(f32) + wq_b.astype(f32) @ wq_a.astype(f32))
    Wk = (wk_w.astype(f32) + wk_b.astype(f32) @ wk_a.astype(f32))
    Wv = (wv_w.astype(f32) + wv_b.astype(f32) @ wv_a.astype(f32))
    Wo = (wo_w.astype(f32) + wo_b.astype(f32) @ wo_a.astype(f32))

    perm = np.concatenate([np.arange(0, HD, 2), np.arange(1, HD, 2)])
    Wq_p = Wq.reshape(H, HD, D)[:, perm, :].reshape(H * HD, D)
    Wk_p = Wk.reshape(KVH, HD, D)[:, perm, :].reshape(KVH * HD, D)

    j = np.arange(HD // 2, dtype=np.float64)
    inv_freq = 1.0 / THETA ** (2.0 * j / HD)
    tpos = np.arange(S, dtype=np.float64)
    freqs = np.outer(inv_freq, tpos)                      # [64, S]
    cosT = np.cos(freqs)
    sinT = np.sin(freqs)
    cos2t = np.concatenate([cosT, cosT], 0).astype(bf16)
    sins2t = np.concatenate([-sinT, sinT], 0).astype(bf16)

    a = np.arange(128)
    triT = (a[:, None] <= a[None, :]).astype(bf16)
    edgeT = (a[:, None] > a[None, :]).astype(bf16)
    edgegT = ((a[:, None] > a[None, :]) | (a[:, None] < GLOBAL)).astype(bf16)
    ident = np.eye(128, dtype=bf16)

    common = dict(cos2t=cos2t, sins2t=sins2t, triT=triT, edgeT=edgeT,
                  edgegT=edgegT, ident=ident)

    def pack_w(wT, nf):
        # [D, nf] -> [128, 16, nf], partition-contiguous
        return np.ascontiguousarray(
            wT.reshape(16, 128, nf).transpose(1, 0, 2)).astype(bf16)

    NCH_ = S // TOK
    in_maps = []
    for b in range(B):
        xT = x[b].astype(f32).T.astype(bf16)            # [D, S]
        xh = np.ascontiguousarray(
            xT.reshape(16, 128, NCH_, TOK).transpose(2, 1, 0, 3))
        for g in range(KVH):
            woT = Wo[:, GF * g:GF * (g + 1)].T          # [GF, D]
            woh = np.ascontiguousarray(
                woT.reshape(GH, 128, D).transpose(1, 0, 2)).astype(bf16)
            in_maps.append(dict(
                xt=xh,
                wqt=pack_w(Wq_p[GF * g:GF * (g + 1), :].T, GF),
                wkt=pack_w(Wk_p[HD * g:HD * (g + 1), :].T, HD),
                wvt=pack_w(Wv[HD * g:HD * (g + 1), :].T, HD),
                wot=woh,
                **common,
            ))
    return in_maps


def kernel(**inputs):
    nc = _build_program()
    in_maps = _host_inputs(**inputs)
    res = None
    last_err = None
    for _attempt in range(4):
        try:
            res = run_bass_kernel_spmd(nc, in_maps,
                                       core_ids=list(range(NCORES)))
            break
        except Exception as e:  # transient first-exec device hiccups
            last_err = e
            import time as _time
            _time.sleep(3.0 * (_attempt + 1))
    if res is None:
        raise last_err
    out = np.zeros((B, S, D), dtype=np.float32)
    for b in range(B):
        for g in range(KVH):
            out[b] += res.results[b * KVH + g]["y"].astype(np.float32)
    return out
